# revision 35
# baseline (speedup 1.0000x reference)
"""Cross-attention kernel for Trainium2 (8 NeuronCores), v4.

Problem (reference semantics, fp32 reference):
    q = split_heads(dec @ q_w + q_b)        # [B,H,Sq,64]
    k = split_heads(enc @ k_w + k_b)        # [B,H,Sk,64]
    v = split_heads(enc @ v_w + v_b)        # [B,H,Sk,64]
    a = softmax(mask(q k^T / 8))
    out = merge_heads(a @ v) @ o_w + o_b    # [B,Sq,1024]
with B=4, Sq=1024, Sk=2048, D=1024, H=16.

Sharding: batch x head-group. Core c handles batch b=c//2 and heads
hg*8..hg*8+8 (hg=c%2). Each core emits a full [1024,1024] partial of
its batch's output; the host sums the two partials per batch and adds
o_b + v_b @ o_w (v-bias folds into the output bias host-side). The k
bias is dropped entirely: q.kb is constant across keys per query, so
it cancels in softmax.

Host-side prep (free): dec/enc transposed (feature-major), everything
bf16, operands packed into [128, *] mega-rows so each tensor arrives
in 1-4 large DMAs (~625ns fixed issue cost per DMA).

Device schedule, built around ScalarE's exp stream (the 133us clock):
  - PASS1: kT[0] per enc quarter sg + ALL hp0 scores (both dec halves)
    emitted per sg; qT[1..3] matmuls interleaved as PE filler between
    exp-gated score pairs. ScalarE saturates from ~10us.
  - PASS2 per sg: kT[1..3], v tiles (ones column per head accumulates
    the softmax denominator Z in PV), eager hp1 scores (both halves),
    and PV beats for sub-phases (0,0) [same-sg trail] and (0,1)
    [one-sg lag] so exp ring slots recycle during the projections.
  - C: sub-phases (1,0)..(3,1): 16 PV beats each + score pops in
    consumption order + lag-1 o-projection partials (128-contraction
    matmul + DVE add into bf16 otacc; last head pair lands in fp32 ot
    tiles DMA'd per dec tile). Norm per (hp,qh): DVE reciprocal of Z,
    gpsimd partition broadcast, DVE multiply into bf16 xT.
  - Exp tiles live in TWO FIFO rings (one per dec half qh): within a
    ring, allocation order (hp asc, c asc) equals PV consumption
    order, so slot reuse never head-blocks or deadlocks.
"""
import numpy as np

P = 128
B, S_ENC, S_DEC, D, H = 4, 2048, 1024, 1024, 16
HD = D // H                     # 64
NCORES = 8
FSH = 512                       # features per core (8 heads x 64)
HPC = 8                         # heads per core
NQT = S_DEC // P                # 8
NET = D // P                    # 8
NST = S_ENC // P                # 16
NFT = FSH // P                  # 4
VG = HD + 1                     # 65: v cols per head incl ones column

_NC = {}


def _build_nc(repeat=1):
    from contextlib import ExitStack
    import concourse.bass as bass
    import concourse.tile as tile
    from concourse import bacc, mybir

    F32 = mybir.dt.float32
    BF16 = mybir.dt.bfloat16
    Exp = mybir.ActivationFunctionType.Exp

    nc = bacc.Bacc("TRN2", target_bir_lowering=False, debug=False)

    decT = nc.dram_tensor("decT", [P, NET * S_DEC], BF16,
                          kind="ExternalInput").ap()
    encT = nc.dram_tensor("encT", [P, NET * S_ENC], BF16,
                          kind="ExternalInput").ap()
    qw = nc.dram_tensor("qw", [P, NET * FSH], BF16, kind="ExternalInput").ap()
    # qw/kw are ft-major ([p, ft*1024 + j*128 + f]) so each qT/kT
    # stripe unblocks on a quarter-size DMA chunk
    kw = nc.dram_tensor("kw", [P, NET * FSH], BF16, kind="ExternalInput").ap()
    vw = nc.dram_tensor("vw", [P, NET * FSH], BF16, kind="ExternalInput").ap()
    ow = nc.dram_tensor("ow", [P, NFT * D], BF16, kind="ExternalInput").ap()
    qb = nc.dram_tensor("qb", [P, NFT], F32, kind="ExternalInput").ap()
    maskb = nc.dram_tensor("maskb", [P, NST], F32, kind="ExternalInput").ap()
    # bf16 output partials: halves the output DMA (host upcasts + sums)
    outp = nc.dram_tensor("outp", [S_DEC, D], BF16,
                          kind="ExternalOutput").ap()

    with tile.TileContext(nc) as tc, ExitStack() as ctx:
        const = ctx.enter_context(tc.tile_pool(name="const", bufs=1))
        qb_t = const.tile([P, NFT], F32, tag="qb")
        maskb_t = const.tile([P, NST], F32, tag="maskb")
        dume = const.tile([1, 1], F32, tag="dume")

        persist = ctx.enter_context(tc.tile_pool(name="persist", bufs=1))
        for rep in range(repeat):
            _emit_rep(nc, tc, persist, rep, rep == 0,
                      decT, encT, qw, kw, vw, ow, qb, maskb, outp,
                      qb_t, maskb_t, dume)

    nc.compile()
    return nc


def _emit_rep(nc, tc, persist, rep, first,
              decT, encT, qw, kw, vw, ow, qb, maskb, outp,
              qb_t, maskb_t, dume):
    import concourse.bass as bass
    from concourse import mybir

    F32 = mybir.dt.float32
    BF16 = mybir.dt.bfloat16
    ts = bass.ts
    Exp = mybir.ActivationFunctionType.Exp
    R = f"r{rep}_" if rep else ""

    qT = [persist.tile([P, S_DEC], BF16, tag=f"qT{t}", name=f"{R}qT{t}")
          for t in range(NFT)]
    kT = [persist.tile([P, S_ENC], BF16, tag=f"kT{t}", name=f"{R}kT{t}")
          for t in range(NFT)]
    vt = [persist.tile([P, HPC * VG], BF16, tag=f"v{t}", name=f"{R}v{t}")
          for t in range(NST)]
    xT = [persist.tile([P, S_DEC], BF16, tag=f"xT{t}", name=f"{R}xT{t}")
          for t in range(NFT)]

    # per-qh exp rings: within each, (hp asc, c asc) allocation order
    # equals PV consumption order
    expq = [tc.tile_pool(name=f"{R}expq{q}", bufs=20) for q in range(2)]
    expp = [cm.__enter__() for cm in expq]
    ex_tiles = {}

    def make_emit_scores(scpool):
        def emit_scores(hp, qh, c):
            """Row-tiled paired scores + fused exp for heads 2hp
            (PE rows 0:64) and 2hp+1 (rows 64:128)."""
            sc = scpool.tile([P, 1024], F32, tag="sc", name="sc")
            for par in range(2):
                r0 = par * HD
                nc.tensor.matmul(sc[:, ts(par, 512)],
                                 kT[hp][r0:r0 + HD, ts(c, P)],
                                 qT[hp][r0:r0 + HD, ts(qh, 512)],
                                 start=True, stop=True)
            ex = expp[qh].tile([P, 1024], BF16, tag="ex", name="ex")
            nc.scalar.activation(ex[:], sc[:], Exp,
                                 bias=maskb_t[:, c:c + 1], scale=0.125)
            ex_tiles[(hp, qh, c)] = ex
        return emit_scores

    def make_emit_scores_half(scpool):
        def emit_scores_half(hp, qh, c):
            """Per-par half tiles ([128,512], 1 bank) so score matmuls
            double-buffer against the exp stream at half the PSUM."""
            ex = expp[qh].tile([P, 1024], BF16, tag="ex", name="ex")
            for par in range(2):
                r0 = par * HD
                sch = scpool.tile([P, 512], F32, tag="sch", name="sch")
                nc.tensor.matmul(sch[:],
                                 kT[hp][r0:r0 + HD, ts(c, P)],
                                 qT[hp][r0:r0 + HD, ts(qh, 512)],
                                 start=True, stop=True)
                nc.scalar.activation(ex[:, ts(par, 512)], sch[:], Exp,
                                     bias=maskb_t[:, c:c + 1],
                                     scale=0.125)
            ex_tiles[(hp, qh, c)] = ex
        return emit_scores_half

    def pv_beat(hp, qh, c, xp):
        ex = ex_tiles.pop((hp, qh, c))
        for par in range(2):
            h = 2 * hp + par
            nc.tensor.matmul(xp[par][:], vt[c][:, h * VG:(h + 1) * VG],
                             ex[:, ts(par, 512)],
                             start=(c == 0), stop=(c == NST - 1))

    squeue = [(hp, qh, c) for hp in range(2, NFT)
              for qh in range(2) for c in range(NST)]
    sq_i = [0]

    with tc.tile_pool(name=f"{R}bw", bufs=1) as bw, \
         tc.tile_pool(name=f"{R}encp", bufs=1) as encp:
        kw_a = bw.tile([P, NET * FSH], BF16, tag="kw", name=f"{R}kw")
        vw_a = bw.tile([P, NET * FSH], BF16, tag="vw", name=f"{R}vw")
        enc_t = [encp.tile([P, NET * 512], BF16, tag=f"encT{s}",
                           name=f"{R}encT{s}") for s in range(4)]

        # ---- stage A + PASS1 -------------------------------------------
        with tc.tile_pool(name=f"{R}aw", bufs=1) as aw, \
             tc.tile_pool(name=f"{R}pqs", bufs=1, space="PSUM") as pqs, \
             tc.tile_pool(name=f"{R}pk0", bufs=2, space="PSUM") as pk0, \
             tc.tile_pool(name=f"{R}sc1", bufs=2, space="PSUM") as sc1:
            emit_scores1 = make_emit_scores(sc1)
            qw_a = aw.tile([P, NET * FSH], BF16, tag="qw", name=f"{R}qw")
            dec_a = aw.tile([P, NET * S_DEC], BF16, tag="decT",
                            name=f"{R}decT")
            FT = NET * P
            # critical-path DMAs first: ft0 weight chunks + dec j-pairs
            # so qT[0]/kT[0] unblock on quarter-size transfers
            nc.sync.dma_start(qw_a[:, 0:FT], qw[:, 0:FT])
            nc.sync.dma_start(kw_a[:, 0:FT], kw[:, 0:FT])
            nc.sync.dma_start(dec_a[:, 0:2 * S_DEC], decT[:, 0:2 * S_DEC])
            nc.sync.dma_start(enc_t[0][:], encT[:, 0:NET * 512])
            if first:
                nc.sync.dma_start(qb_t[:], qb[:])
                nc.sync.dma_start(maskb_t[:], maskb[:])
                # dummy exp so the act table loads under the DMA wait
                nc.scalar.activation(dume[:], qb_t[0:1, 0:1], Exp)
            nc.sync.dma_start(dec_a[:, 2 * S_DEC:4 * S_DEC],
                              decT[:, 2 * S_DEC:4 * S_DEC])
            nc.sync.dma_start(qw_a[:, FT:], qw[:, FT:])
            nc.sync.dma_start(dec_a[:, 4 * S_DEC:6 * S_DEC],
                              decT[:, 4 * S_DEC:6 * S_DEC])
            nc.sync.dma_start(dec_a[:, 6 * S_DEC:], decT[:, 6 * S_DEC:])
            nc.sync.dma_start(kw_a[:, FT:], kw[:, FT:])
            for s in range(1, 4):
                nc.sync.dma_start(enc_t[s][:],
                                  encT[:, s * NET * 512:(s + 1) * NET * 512])
            nc.sync.dma_start(vw_a[:], vw[:])

            # ones column per head in the v tiles (Z accumulator)
            for t in range(NST):
                dst = vt[t][:].rearrange("p (h c) -> p h c",
                                         h=HPC, c=VG)[:, :, HD:VG]
                nc.gpsimd.memset(dst, 1.0)

            def qt_mms(ft, pq, j0, j1):
                # matmul PSUM outputs must stay within one 2KB bank ->
                # 512-wide halves
                for j in range(j0, j1):
                    for h in range(2):
                        hs = bass.ts(h, 512)
                        nc.tensor.matmul(pq[:, hs],
                                         qw_a[:, ft * NET * P + j * P:
                                              ft * NET * P + (j + 1) * P],
                                         dec_a[:, j * S_DEC:(j + 1) * S_DEC]
                                         [:, hs],
                                         start=(j == 0), stop=(j == NET - 1))

            # qT[0] straight through
            pq = pqs.tile([P, S_DEC], F32, tag="pq")
            qt_mms(0, pq, 0, NET)
            nc.vector.tensor_scalar_add(qT[0][:], pq[:], qb_t[:, 0:1])

            # PASS1: per sg: kT[0] chunk + hp0 scores (both qh), with
            # qT[sg+1] matmuls as filler between exp-gated score pairs
            for sg in range(4):
                pk = pk0.tile([P, 512], F32, tag="pk")
                for j in range(NET):
                    nc.tensor.matmul(pk[:],
                                     kw_a[:, j * P:(j + 1) * P],
                                     enc_t[sg][:, j * 512:(j + 1) * 512],
                                     start=(j == 0), stop=(j == NET - 1))
                nc.vector.tensor_copy(kT[0][:, ts(sg, 512)], pk[:])
                nxt = sg + 1 if sg < 3 else None
                if nxt is not None:
                    pq = pqs.tile([P, S_DEC], F32, tag="pq")
                for i, cc in enumerate(range(sg * 4, sg * 4 + 4)):
                    emit_scores1(0, 0, cc)
                    if nxt is not None:
                        qt_mms(nxt, pq, 2 * i, 2 * i + 1)
                    emit_scores1(0, 1, cc)
                    if nxt is not None:
                        qt_mms(nxt, pq, 2 * i + 1, 2 * i + 2)
                if nxt is not None:
                    nc.vector.tensor_scalar_add(qT[nxt][:], pq[:],
                                                qb_t[:, nxt:nxt + 1])

        # ---- PASS2 ------------------------------------------------------
        xpb_cm = tc.tile_pool(name=f"{R}xpb", bufs=4, space="PSUM")
        xpb = xpb_cm.__enter__()
        xp00 = [xpb.tile([VG, 512], F32, tag="xpb", name="xp00")
                for _ in range(2)]
        xp01 = [xpb.tile([VG, 512], F32, tag="xpb", name="xp01")
                for _ in range(2)]

        with tc.tile_pool(name=f"{R}pks", bufs=1, space="PSUM") as pks, \
             tc.tile_pool(name=f"{R}pvs", bufs=1, space="PSUM") as pvs, \
             tc.tile_pool(name=f"{R}sc2", bufs=2, space="PSUM") as sc2:
            emit_scores2 = make_emit_scores_half(sc2)
            for sg in range(4):
                cb = sg * 4
                for ft in (1, 2, 3):
                    pk = pks.tile([P, 512], F32, tag="pk")
                    for j in range(NET):
                        nc.tensor.matmul(pk[:],
                                         kw_a[:, ft * NET * P + j * P:
                                              ft * NET * P + (j + 1) * P],
                                         enc_t[sg][:, j * 512:(j + 1) * 512],
                                         start=(j == 0), stop=(j == NET - 1))
                    nc.vector.tensor_copy(kT[ft][:, ts(sg, 512)], pk[:])
                    # hp1/qh0 scores spread across the ft groups (two per
                    # group) so the 1-buf sc pool never head-blocks PE
                    if ft >= 2:
                        for cc in range(cb + 2 * (ft - 2),
                                        cb + 2 * (ft - 1)):
                            emit_scores2(1, 0, cc)
                for st in range(4):
                    pv = pvs.tile([P, 512], F32, tag="pv")
                    for j in range(NET):
                        nc.tensor.matmul(pv[:],
                                         enc_t[sg][:, j * 512 + st * P:
                                               j * 512 + (st + 1) * P],
                                         vw_a[:, j * FSH:(j + 1) * FSH],
                                         start=(j == 0), stop=(j == NET - 1))
                    dst = vt[cb + st][:].rearrange(
                        "p (h c) -> p h c", h=HPC, c=VG)[:, :, 0:HD]
                    nc.vector.tensor_copy(
                        dst, pv[:].rearrange("p (h c) -> p h c",
                                             h=HPC, c=HD))
                    pv_beat(0, 0, cb + st, xp00)
                    if cb + st - 4 >= 0:
                        pv_beat(0, 1, cb + st - 4, xp01)
                    # hp1/qh1 score per st step: its ring slot reuses the
                    # (0,1,c) slot freed by the lag-4 PV beat just above
                    emit_scores2(1, 1, cb + st)


        # ---- C-head: finish sub-phase (0,1), norms for hp0 --------------
        with tc.tile_pool(name=f"{R}zph", bufs=2) as zph, \
             tc.tile_pool(name=f"{R}scph", bufs=2, space="PSUM") as scph:
            emit_scores_h = make_emit_scores(scph)
            for i, c in enumerate(range(12, NST)):
                emit_scores_h(*squeue[i])
                pv_beat(0, 1, c, xp01)
            sq_i[0] = 4
            for hp, xp, qh in ((0, xp00, 0), (0, xp01, 1)):
                qs = ts(qh, 512)
                for par in range(2):
                    r0 = par * HD
                    zrec = zph.tile([1, 512], F32, tag="zrec", name="zrec")
                    nc.vector.reciprocal(zrec[:], xp[par][HD:VG, :])
                    zbs = zph.tile([HD, 512], F32, tag="zbs", name="zbs")
                    nc.gpsimd.partition_broadcast(zbs[:], zrec[:])
                    nc.vector.tensor_mul(xT[hp][r0:r0 + HD, qs],
                                         xp[par][0:HD, :], zbs[:])
        xpb_cm.__exit__(None, None, None)

    # ---- stages C+D (enc/kw/vw released) --------------------------------
    with tc.tile_pool(name=f"{R}cpool", bufs=1) as cpool, \
         tc.tile_pool(name=f"{R}zp", bufs=4) as zp, \
         tc.tile_pool(name=f"{R}otp", bufs=2) as otp:
        ow_a = cpool.tile([P, NFT * D], BF16, tag="ow", name=f"{R}ow")
        otacc = [cpool.tile([P, D], BF16, tag=f"oa{t}", name=f"{R}oa{t}")
                 for t in range(NQT)]
        nc.sync.dma_start(ow_a[:], ow[:])
        ot_cur = [None]

        def emit_norm(hp, xp, qh):
            qs = ts(qh, 512)
            for par in range(2):
                r0 = par * HD
                zrec = zp.tile([1, 512], F32, tag="zrec", name="zrec")
                nc.vector.reciprocal(zrec[:], xp[par][HD:VG, :])
                zbs = zp.tile([HD, 512], F32, tag="zbs", name="zbs")
                nc.gpsimd.partition_broadcast(zbs[:], zrec[:])
                nc.vector.tensor_mul(xT[hp][r0:r0 + HD, qs],
                                     xp[par][0:HD, :], zbs[:])

        def make_opart(pos):
            def emit_opart_one(hp, qt, gh):
                po = pos.tile([P, 512], F32, tag="po")
                nc.tensor.matmul(po[:], xT[hp][:, ts(qt, P)],
                                 ow_a[:, hp * D + gh * 512:
                                      hp * D + (gh + 1) * 512],
                                 start=True, stop=True)
                gs = ts(gh, 512)
                if hp == 0:
                    nc.vector.tensor_copy(otacc[qt][:, gs], po[:])
                elif hp < NFT - 1:
                    nc.vector.tensor_add(otacc[qt][:, gs],
                                         otacc[qt][:, gs], po[:])
                else:
                    if gh == 0:
                        ot_cur[0] = otp.tile([P, D], BF16, tag="ot",
                                             name="ot")
                    nc.vector.tensor_add(ot_cur[0][:, gs],
                                         otacc[qt][:, gs], po[:])
                    if gh == 1:
                        nc.sync.dma_start(outp[ts(qt, P), :], ot_cur[0][:])
            return emit_opart_one

        backlog = [(0, 0), (0, 1)]

        def make_pv_phase(xps, opart, pop):
            def emit_pv_phase(hp, qh):
                oparts = []
                take = 2 if len(backlog) >= 2 else len(backlog)
                for _ in range(take):
                    ph, pq_ = backlog.pop(0)
                    oparts += [(ph, qt, gh)
                               for qt in range(pq_ * 4, pq_ * 4 + 4)
                               for gh in range(2)]
                xp = [xps.tile([VG, 512], F32, tag="xp", name="xp")
                      for _ in range(2)]
                for c in range(NST):
                    pv_beat(hp, qh, c, xp)
                    if oparts:
                        opart(*oparts.pop(0))
                    pop()
                emit_norm(hp, xp, qh)
                backlog.append((hp, qh))
            return emit_pv_phase

        # sub-phases (1,0)..(2,1): score pops still flowing
        with tc.tile_pool(name=f"{R}scps3", bufs=2, space="PSUM") as scps3:
            emit_scores3 = make_emit_scores(scps3)

            def pop_scores():
                if sq_i[0] < len(squeue):
                    hp, qh, c = squeue[sq_i[0]]
                    sq_i[0] += 1
                    emit_scores3(hp, qh, c)

            with tc.tile_pool(name=f"{R}xps", bufs=3,
                              space="PSUM") as xps, \
                 tc.tile_pool(name=f"{R}pos", bufs=1,
                              space="PSUM") as pos:
                emit_pv_phase = make_pv_phase(xps, make_opart(pos),
                                              pop_scores)
                for hp, qh in ((1, 0), (1, 1), (2, 0), (2, 1)):
                    emit_pv_phase(hp, qh)
            assert sq_i[0] >= len(squeue), "score queue must drain"

        # sub-phases (3,0),(3,1) + opart drain: queue is dry, so the
        # score-pool banks become a deeper opart pipeline
        with tc.tile_pool(name=f"{R}xps2", bufs=4, space="PSUM") as xps2, \
             tc.tile_pool(name=f"{R}pos2", bufs=3, space="PSUM") as pos2:
            opart2 = make_opart(pos2)
            emit_pv_phase2 = make_pv_phase(xps2, opart2, lambda: None)
            for hp, qh in ((3, 0), (3, 1)):
                emit_pv_phase2(hp, qh)
            while backlog:
                ph, pq_ = backlog.pop(0)
                for qt in range(pq_ * 4, pq_ * 4 + 4):
                    for gh in range(2):
                        opart2(ph, qt, gh)
    for cm in reversed(expq):
        cm.__exit__(None, None, None)


def _get_nc(repeat=1):
    if repeat not in _NC:
        _NC[repeat] = _build_nc(repeat)
    return _NC[repeat]


def _mega(x, nblk, bf16):
    """[nblk*128, F] -> [128, nblk*F] with block-major free dim."""
    nb, f = nblk, x.shape[1]
    return np.ascontiguousarray(
        x.reshape(nb, P, f).transpose(1, 0, 2).reshape(P, nb * f)
        .astype(bf16))


def _mega_ft(x, bf16):
    """[NET*128, NFT*128] -> [128, NFT*NET*128], ft-major then j."""
    x4 = x.reshape(NET, P, NFT, P).transpose(1, 2, 0, 3)
    return np.ascontiguousarray(
        x4.reshape(P, NFT * NET * P).astype(bf16))


def make_in_maps(enc, enc_mask, dec, q_w, q_b, k_w, k_b, v_w, v_b, o_w, o_b):
    import ml_dtypes
    bf16 = ml_dtypes.bfloat16
    f32 = np.float32
    ca = np.ascontiguousarray
    in_maps = []
    decT_b, encT_b = [], []
    for b in range(B):
        dT = np.asarray(dec[b], dtype=f32).T          # [1024, 1024]
        decT_b.append(_mega(dT, NET, bf16))
        eT = np.asarray(enc[b], dtype=f32).T          # [1024, 2048]
        # sg-major, then j within: [128, 4*8*512]
        e4 = eT.reshape(NET, P, 4, 512).transpose(1, 2, 0, 3)
        encT_b.append(ca(e4.reshape(P, NET * S_ENC).astype(bf16)))
    for c in range(NCORES):
        b, hg = c // 2, c % 2
        fs = slice(hg * FSH, (hg + 1) * FSH)
        mb = np.where(np.asarray(enc_mask[b, 0, 0]), f32(-1e30), f32(0.0))
        in_maps.append({
            "decT": decT_b[b],
            "encT": encT_b[b],
            "qw": _mega_ft(np.asarray(q_w[:, fs], dtype=f32), bf16),
            "kw": _mega_ft(np.asarray(k_w[:, fs], dtype=f32), bf16),
            "vw": _mega(np.asarray(v_w[:, fs], dtype=f32), NET, bf16),
            "ow": _mega(np.asarray(o_w[fs, :], dtype=f32), NFT, bf16),
            "qb": ca(np.asarray(q_b[fs], dtype=f32).reshape(NFT, P).T),
            "maskb": ca(mb.astype(f32).reshape(NST, P).T),
        })
    return in_maps


def assemble(results, o_b, v_b, o_w):
    ob_eff = (np.asarray(o_b, dtype=np.float64)
              + np.asarray(v_b, dtype=np.float64)
              @ np.asarray(o_w, dtype=np.float64)).astype(np.float32)
    out = np.empty((B, S_DEC, D), dtype=np.float32)
    for b in range(B):
        out[b] = (np.asarray(results[2 * b]["outp"], dtype=np.float32)
                  + np.asarray(results[2 * b + 1]["outp"], dtype=np.float32)
                  + ob_eff)
    return out


def kernel(enc, enc_mask, dec, q_w, q_b, k_w, k_b, v_w, v_b, o_w, o_b):
    from concourse.bass_utils import run_bass_kernel_spmd
    nc = _get_nc()
    in_maps = make_in_maps(enc, enc_mask, dec, q_w, q_b, k_w, k_b,
                           v_w, v_b, o_w, o_b)
    res = run_bass_kernel_spmd(nc, in_maps, list(range(NCORES)))
    return assemble(res.results, o_b, v_b, o_w)


# revision 37
# speedup vs baseline: 1.5057x; 1.5057x over previous
"""Cross-attention kernel for Trainium2 (8 NeuronCores), v4.

Problem (reference semantics, fp32 reference):
    q = split_heads(dec @ q_w + q_b)        # [B,H,Sq,64]
    k = split_heads(enc @ k_w + k_b)        # [B,H,Sk,64]
    v = split_heads(enc @ v_w + v_b)        # [B,H,Sk,64]
    a = softmax(mask(q k^T / 8))
    out = merge_heads(a @ v) @ o_w + o_b    # [B,Sq,1024]
with B=4, Sq=1024, Sk=2048, D=1024, H=16.

Sharding: batch x head-group. Core c handles batch b=c//2 and heads
hg*8..hg*8+8 (hg=c%2). Each core emits a full [1024,1024] partial of
its batch's output; the host sums the two partials per batch and adds
o_b + v_b @ o_w (v-bias folds into the output bias host-side). The k
bias is dropped entirely: q.kb is constant across keys per query, so
it cancels in softmax.

Host-side prep (free): dec/enc transposed (feature-major), everything
bf16, operands packed into [128, *] mega-rows so each tensor arrives
in 1-4 large DMAs (~625ns fixed issue cost per DMA).

Device schedule, built around ScalarE's exp stream (the 133us clock):
  - PASS1: kT[0] per enc quarter sg + ALL hp0 scores (both dec halves)
    emitted per sg; qT[1..3] matmuls interleaved as PE filler between
    exp-gated score pairs. ScalarE saturates from ~10us.
  - PASS2 per sg: kT[1..3], v tiles (ones column per head accumulates
    the softmax denominator Z in PV), eager hp1 scores (both halves),
    and PV beats for sub-phases (0,0) [same-sg trail] and (0,1)
    [one-sg lag] so exp ring slots recycle during the projections.
  - C: sub-phases (1,0)..(3,1): 16 PV beats each + score pops in
    consumption order + lag-1 o-projection partials (128-contraction
    matmul + DVE add into bf16 otacc; last head pair lands in fp32 ot
    tiles DMA'd per dec tile). Norm per (hp,qh): DVE reciprocal of Z,
    gpsimd partition broadcast, DVE multiply into bf16 xT.
  - Exp tiles live in TWO FIFO rings (one per dec half qh): within a
    ring, allocation order (hp asc, c asc) equals PV consumption
    order, so slot reuse never head-blocks or deadlocks.
"""
import numpy as np

P = 128
B, S_ENC, S_DEC, D, H = 4, 2048, 1024, 1024, 16
HD = D // H                     # 64
NCORES = 8
FSH = 512                       # features per core (8 heads x 64)
HPC = 8                         # heads per core
NQT = S_DEC // P                # 8
NET = D // P                    # 8
NST = S_ENC // P                # 16
NFT = FSH // P                  # 4
VG = HD + 1                     # 65: v cols per head incl ones column

_NC = {}


def _build_nc(repeat=1):
    from contextlib import ExitStack
    import concourse.bass as bass
    import concourse.tile as tile
    from concourse import bacc, mybir

    F32 = mybir.dt.float32
    BF16 = mybir.dt.bfloat16
    Exp = mybir.ActivationFunctionType.Exp

    nc = bacc.Bacc("TRN2", target_bir_lowering=False, debug=False)

    decT = nc.dram_tensor("decT", [P, NET * S_DEC], BF16,
                          kind="ExternalInput").ap()
    encT = nc.dram_tensor("encT", [P, NET * S_ENC], BF16,
                          kind="ExternalInput").ap()
    qw = nc.dram_tensor("qw", [P, NET * FSH], BF16, kind="ExternalInput").ap()
    # qw/kw are ft-major ([p, ft*1024 + j*128 + f]) so each qT/kT
    # stripe unblocks on a quarter-size DMA chunk
    kw = nc.dram_tensor("kw", [P, NET * FSH], BF16, kind="ExternalInput").ap()
    vw = nc.dram_tensor("vw", [P, NET * FSH], BF16, kind="ExternalInput").ap()
    ow = nc.dram_tensor("ow", [P, NFT * D], BF16, kind="ExternalInput").ap()
    qb = nc.dram_tensor("qb", [P, NFT], F32, kind="ExternalInput").ap()
    maskb = nc.dram_tensor("maskb", [P, NST], F32, kind="ExternalInput").ap()
    # bf16 output partials: halves the output DMA (host upcasts + sums)
    outp = nc.dram_tensor("outp", [S_DEC, D], BF16,
                          kind="ExternalOutput").ap()

    with tile.TileContext(nc) as tc, ExitStack() as ctx:
        const = ctx.enter_context(tc.tile_pool(name="const", bufs=1))
        qb_t = const.tile([P, NFT], F32, tag="qb")
        maskb_t = const.tile([P, NST], F32, tag="maskb")
        dume = const.tile([1, 1], F32, tag="dume")

        persist = ctx.enter_context(tc.tile_pool(name="persist", bufs=1))
        for rep in range(repeat):
            _emit_rep(nc, tc, persist, rep, rep == 0,
                      decT, encT, qw, kw, vw, ow, qb, maskb, outp,
                      qb_t, maskb_t, dume)

    nc.compile()
    return nc


def _emit_rep(nc, tc, persist, rep, first,
              decT, encT, qw, kw, vw, ow, qb, maskb, outp,
              qb_t, maskb_t, dume):
    import concourse.bass as bass
    from concourse import mybir

    F32 = mybir.dt.float32
    BF16 = mybir.dt.bfloat16
    ts = bass.ts
    Exp = mybir.ActivationFunctionType.Exp
    R = f"r{rep}_" if rep else ""

    qT = [persist.tile([P, S_DEC], BF16, tag=f"qT{t}", name=f"{R}qT{t}")
          for t in range(NFT)]
    kT = [persist.tile([P, S_ENC], BF16, tag=f"kT{t}", name=f"{R}kT{t}")
          for t in range(NFT)]
    vt = [persist.tile([P, HPC * VG], BF16, tag=f"v{t}", name=f"{R}v{t}")
          for t in range(NST)]
    xT = [persist.tile([P, S_DEC], BF16, tag=f"xT{t}", name=f"{R}xT{t}")
          for t in range(NFT)]

    # per-qh exp rings: within each, (hp asc, c asc) allocation order
    # equals PV consumption order
    expq = [tc.tile_pool(name=f"{R}expq{q}", bufs=20) for q in range(2)]
    expp = [cm.__enter__() for cm in expq]
    ex_tiles = {}

    def make_emit_scores(scpool):
        def emit_scores(hp, qh, c):
            """Row-tiled paired scores + fused exp for heads 2hp
            (PE rows 0:64) and 2hp+1 (rows 64:128)."""
            sc = scpool.tile([P, 1024], F32, tag="sc", name="sc")
            for par in range(2):
                r0 = par * HD
                nc.tensor.matmul(sc[:, ts(par, 512)],
                                 kT[hp][r0:r0 + HD, ts(c, P)],
                                 qT[hp][r0:r0 + HD, ts(qh, 512)],
                                 start=True, stop=True)
            ex = expp[qh].tile([P, 1024], BF16, tag="ex", name="ex")
            nc.scalar.activation(ex[:], sc[:], Exp,
                                 bias=maskb_t[:, c:c + 1], scale=0.125)
            ex_tiles[(hp, qh, c)] = ex
        return emit_scores

    def make_emit_scores_half(scpool):
        def emit_scores_half(hp, qh, c):
            """Per-par half tiles ([128,512], 1 bank) so score matmuls
            double-buffer against the exp stream at half the PSUM."""
            ex = expp[qh].tile([P, 1024], BF16, tag="ex", name="ex")
            for par in range(2):
                r0 = par * HD
                sch = scpool.tile([P, 512], F32, tag="sch", name="sch")
                nc.tensor.matmul(sch[:],
                                 kT[hp][r0:r0 + HD, ts(c, P)],
                                 qT[hp][r0:r0 + HD, ts(qh, 512)],
                                 start=True, stop=True)
                nc.scalar.activation(ex[:, ts(par, 512)], sch[:], Exp,
                                     bias=maskb_t[:, c:c + 1],
                                     scale=0.125)
            ex_tiles[(hp, qh, c)] = ex
        return emit_scores_half

    def pv_beat(hp, qh, c, xp):
        ex = ex_tiles.pop((hp, qh, c))
        for par in range(2):
            h = 2 * hp + par
            nc.tensor.matmul(xp[par][:], vt[c][:, h * VG:(h + 1) * VG],
                             ex[:, ts(par, 512)],
                             start=(c == 0), stop=(c == NST - 1))

    squeue = [(hp, qh, c) for hp in range(2, NFT)
              for qh in range(2) for c in range(NST)]
    sq_i = [0]

    with tc.tile_pool(name=f"{R}bw", bufs=1) as bw, \
         tc.tile_pool(name=f"{R}encp", bufs=1) as encp:
        kw_a = bw.tile([P, NET * FSH], BF16, tag="kw", name=f"{R}kw")
        vw_a = bw.tile([P, NET * FSH], BF16, tag="vw", name=f"{R}vw")
        enc_t = [encp.tile([P, NET * 512], BF16, tag=f"encT{s}",
                           name=f"{R}encT{s}") for s in range(4)]

        # ---- stage A + PASS1 -------------------------------------------
        with tc.tile_pool(name=f"{R}aw", bufs=1) as aw, \
             tc.tile_pool(name=f"{R}pqs", bufs=1, space="PSUM") as pqs, \
             tc.tile_pool(name=f"{R}pk0", bufs=2, space="PSUM") as pk0, \
             tc.tile_pool(name=f"{R}sc1", bufs=2, space="PSUM") as sc1:
            emit_scores1 = make_emit_scores(sc1)
            qw_a = aw.tile([P, NET * FSH], BF16, tag="qw", name=f"{R}qw")
            dec_a = aw.tile([P, NET * S_DEC], BF16, tag="decT",
                            name=f"{R}decT")
            FT = NET * P
            # critical-path DMAs first: ft0 weight chunks + dec j-pairs
            # so qT[0]/kT[0] unblock on quarter-size transfers
            nc.sync.dma_start(qw_a[:, 0:FT], qw[:, 0:FT])
            nc.sync.dma_start(kw_a[:, 0:FT], kw[:, 0:FT])
            nc.sync.dma_start(dec_a[:, 0:2 * S_DEC], decT[:, 0:2 * S_DEC])
            nc.sync.dma_start(enc_t[0][:], encT[:, 0:NET * 512])
            if first:
                nc.sync.dma_start(qb_t[:], qb[:])
                nc.sync.dma_start(maskb_t[:], maskb[:])
                # dummy exp so the act table loads under the DMA wait
                nc.scalar.activation(dume[:], qb_t[0:1, 0:1], Exp)
            nc.sync.dma_start(dec_a[:, 2 * S_DEC:4 * S_DEC],
                              decT[:, 2 * S_DEC:4 * S_DEC])
            nc.sync.dma_start(qw_a[:, FT:], qw[:, FT:])
            nc.sync.dma_start(dec_a[:, 4 * S_DEC:6 * S_DEC],
                              decT[:, 4 * S_DEC:6 * S_DEC])
            nc.sync.dma_start(dec_a[:, 6 * S_DEC:], decT[:, 6 * S_DEC:])
            nc.sync.dma_start(kw_a[:, FT:], kw[:, FT:])
            for s in range(1, 4):
                nc.sync.dma_start(enc_t[s][:],
                                  encT[:, s * NET * 512:(s + 1) * NET * 512])
            nc.sync.dma_start(vw_a[:], vw[:])

            # ones column per head in the v tiles (Z accumulator)
            for t in range(NST):
                dst = vt[t][:].rearrange("p (h c) -> p h c",
                                         h=HPC, c=VG)[:, :, HD:VG]
                nc.gpsimd.memset(dst, 1.0)

            def qt_mms(ft, pq, j0, j1):
                # matmul PSUM outputs must stay within one 2KB bank ->
                # 512-wide halves
                for j in range(j0, j1):
                    for h in range(2):
                        hs = bass.ts(h, 512)
                        nc.tensor.matmul(pq[:, hs],
                                         qw_a[:, ft * NET * P + j * P:
                                              ft * NET * P + (j + 1) * P],
                                         dec_a[:, j * S_DEC:(j + 1) * S_DEC]
                                         [:, hs],
                                         start=(j == 0), stop=(j == NET - 1))

            # qT[0] straight through
            pq = pqs.tile([P, S_DEC], F32, tag="pq")
            qt_mms(0, pq, 0, NET)
            nc.vector.tensor_scalar_add(qT[0][:], pq[:], qb_t[:, 0:1])

            # PASS1: per sg: kT[0] chunk + hp0 scores (both qh), with
            # qT[sg+1] matmuls as filler between exp-gated score pairs
            for sg in range(4):
                pk = pk0.tile([P, 512], F32, tag="pk")
                for j in range(NET):
                    nc.tensor.matmul(pk[:],
                                     kw_a[:, j * P:(j + 1) * P],
                                     enc_t[sg][:, j * 512:(j + 1) * 512],
                                     start=(j == 0), stop=(j == NET - 1))
                nc.vector.tensor_copy(kT[0][:, ts(sg, 512)], pk[:])
                nxt = sg + 1 if sg < 3 else None
                if nxt is not None:
                    pq = pqs.tile([P, S_DEC], F32, tag="pq")

                def kt_sg0_mms(ft, pk, j0, j1):
                    # sg3 filler: PASS2-sg0's kT[1]/kT[2] chunks pulled
                    # forward so sg3's exp-gated scores have PE work
                    # behind them
                    for j in range(j0, j1):
                        nc.tensor.matmul(pk[:],
                                         kw_a[:, ft * NET * P + j * P:
                                              ft * NET * P + (j + 1) * P],
                                         enc_t[0][:, j * 512:(j + 1) * 512],
                                         start=(j == 0),
                                         stop=(j == NET - 1))

                if nxt is None:
                    pkf = [pk0.tile([P, 512], F32, tag="pk", name="pkf")
                           for _ in range(2)]
                for i, cc in enumerate(range(sg * 4, sg * 4 + 4)):
                    emit_scores1(0, 0, cc)
                    if nxt is not None:
                        qt_mms(nxt, pq, 2 * i, 2 * i + 1)
                    else:
                        kt_sg0_mms(1 + i // 2, pkf[i // 2],
                                   4 * (i % 2), 4 * (i % 2) + 2)
                    emit_scores1(0, 1, cc)
                    if nxt is not None:
                        qt_mms(nxt, pq, 2 * i + 1, 2 * i + 2)
                    else:
                        kt_sg0_mms(1 + i // 2, pkf[i // 2],
                                   4 * (i % 2) + 2, 4 * (i % 2) + 4)
                        if i % 2 == 1:
                            nc.vector.tensor_copy(
                                kT[1 + i // 2][:, 0:512], pkf[i // 2][:])
                if nxt is not None:
                    nc.vector.tensor_scalar_add(qT[nxt][:], pq[:],
                                                qb_t[:, nxt:nxt + 1])

        # ---- PASS2 ------------------------------------------------------
        xpb_cm = tc.tile_pool(name=f"{R}xpb", bufs=4, space="PSUM")
        xpb = xpb_cm.__enter__()
        xp00 = [xpb.tile([VG, 512], F32, tag="xpb", name="xp00")
                for _ in range(2)]
        xp01 = [xpb.tile([VG, 512], F32, tag="xpb", name="xp01")
                for _ in range(2)]

        with tc.tile_pool(name=f"{R}pks", bufs=1, space="PSUM") as pks, \
             tc.tile_pool(name=f"{R}pvs", bufs=1, space="PSUM") as pvs, \
             tc.tile_pool(name=f"{R}sc2", bufs=2, space="PSUM") as sc2:
            emit_scores2 = make_emit_scores_half(sc2)
            for sg in range(4):
                cb = sg * 4
                for ft in ((3,) if sg == 0 else (1, 2, 3)):
                    pk = pks.tile([P, 512], F32, tag="pk")
                    for j in range(NET):
                        nc.tensor.matmul(pk[:],
                                         kw_a[:, ft * NET * P + j * P:
                                              ft * NET * P + (j + 1) * P],
                                         enc_t[sg][:, j * 512:(j + 1) * 512],
                                         start=(j == 0), stop=(j == NET - 1))
                    nc.vector.tensor_copy(kT[ft][:, ts(sg, 512)], pk[:])
                    # hp1/qh0 scores spread across the ft groups (two per
                    # group) so the 1-buf sc pool never head-blocks PE
                    if sg == 0 and ft == 3:
                        for cc in range(cb, cb + 4):
                            emit_scores2(1, 0, cc)
                    elif sg > 0 and ft >= 2:
                        for cc in range(cb + 2 * (ft - 2),
                                        cb + 2 * (ft - 1)):
                            emit_scores2(1, 0, cc)
                for st in range(4):
                    pv = pvs.tile([P, 512], F32, tag="pv")
                    for j in range(NET):
                        nc.tensor.matmul(pv[:],
                                         enc_t[sg][:, j * 512 + st * P:
                                               j * 512 + (st + 1) * P],
                                         vw_a[:, j * FSH:(j + 1) * FSH],
                                         start=(j == 0), stop=(j == NET - 1))
                    dst = vt[cb + st][:].rearrange(
                        "p (h c) -> p h c", h=HPC, c=VG)[:, :, 0:HD]
                    nc.vector.tensor_copy(
                        dst, pv[:].rearrange("p (h c) -> p h c",
                                             h=HPC, c=HD))
                    pv_beat(0, 0, cb + st, xp00)
                    if cb + st - 4 >= 0:
                        pv_beat(0, 1, cb + st - 4, xp01)
                    # hp1/qh1 score per st step: its ring slot reuses the
                    # (0,1,c) slot freed by the lag-4 PV beat just above
                    emit_scores2(1, 1, cb + st)


        # ---- C-head: finish sub-phase (0,1), norms for hp0 --------------
        with tc.tile_pool(name=f"{R}zph", bufs=2) as zph, \
             tc.tile_pool(name=f"{R}scph", bufs=2, space="PSUM") as scph:
            emit_scores_h = make_emit_scores(scph)
            for i, c in enumerate(range(12, NST)):
                emit_scores_h(*squeue[i])
                pv_beat(0, 1, c, xp01)
            sq_i[0] = 4
            for hp, xp, qh in ((0, xp00, 0), (0, xp01, 1)):
                qs = ts(qh, 512)
                for par in range(2):
                    r0 = par * HD
                    zrec = zph.tile([1, 512], F32, tag="zrec", name="zrec")
                    nc.vector.reciprocal(zrec[:], xp[par][HD:VG, :])
                    zbs = zph.tile([HD, 512], F32, tag="zbs", name="zbs")
                    nc.gpsimd.partition_broadcast(zbs[:], zrec[:])
                    nc.vector.tensor_mul(xT[hp][r0:r0 + HD, qs],
                                         xp[par][0:HD, :], zbs[:])
        xpb_cm.__exit__(None, None, None)

    # ---- stages C+D (enc/kw/vw released) --------------------------------
    with tc.tile_pool(name=f"{R}cpool", bufs=1) as cpool, \
         tc.tile_pool(name=f"{R}zp", bufs=4) as zp, \
         tc.tile_pool(name=f"{R}otp", bufs=2) as otp:
        ow_a = cpool.tile([P, NFT * D], BF16, tag="ow", name=f"{R}ow")
        otacc = [cpool.tile([P, D], BF16, tag=f"oa{t}", name=f"{R}oa{t}")
                 for t in range(NQT)]
        nc.sync.dma_start(ow_a[:], ow[:])
        ot_cur = [None]

        def emit_norm(hp, xp, qh):
            qs = ts(qh, 512)
            for par in range(2):
                r0 = par * HD
                zrec = zp.tile([1, 512], F32, tag="zrec", name="zrec")
                nc.vector.reciprocal(zrec[:], xp[par][HD:VG, :])
                zbs = zp.tile([HD, 512], F32, tag="zbs", name="zbs")
                nc.gpsimd.partition_broadcast(zbs[:], zrec[:])
                nc.vector.tensor_mul(xT[hp][r0:r0 + HD, qs],
                                     xp[par][0:HD, :], zbs[:])

        def make_opart(pos):
            def emit_opart_one(hp, qt, gh):
                po = pos.tile([P, 512], F32, tag="po")
                nc.tensor.matmul(po[:], xT[hp][:, ts(qt, P)],
                                 ow_a[:, hp * D + gh * 512:
                                      hp * D + (gh + 1) * 512],
                                 start=True, stop=True)
                gs = ts(gh, 512)
                if hp == 0:
                    nc.vector.tensor_copy(otacc[qt][:, gs], po[:])
                elif hp < NFT - 1:
                    nc.vector.tensor_add(otacc[qt][:, gs],
                                         otacc[qt][:, gs], po[:])
                else:
                    if gh == 0:
                        ot_cur[0] = otp.tile([P, D], BF16, tag="ot",
                                             name="ot")
                    nc.vector.tensor_add(ot_cur[0][:, gs],
                                         otacc[qt][:, gs], po[:])
                    if gh == 1:
                        nc.sync.dma_start(outp[ts(qt, P), :], ot_cur[0][:])
            return emit_opart_one

        backlog = [(0, 0), (0, 1)]

        def make_pv_phase(xps, opart, pop):
            def emit_pv_phase(hp, qh):
                oparts = []
                take = 2 if len(backlog) >= 2 else len(backlog)
                for _ in range(take):
                    ph, pq_ = backlog.pop(0)
                    oparts += [(ph, qt, gh)
                               for qt in range(pq_ * 4, pq_ * 4 + 4)
                               for gh in range(2)]
                xp = [xps.tile([VG, 512], F32, tag="xp", name="xp")
                      for _ in range(2)]
                for c in range(NST):
                    pv_beat(hp, qh, c, xp)
                    if oparts:
                        opart(*oparts.pop(0))
                    pop()
                emit_norm(hp, xp, qh)
                backlog.append((hp, qh))
            return emit_pv_phase

        # sub-phases (1,0)..(2,1): score pops still flowing
        with tc.tile_pool(name=f"{R}scps3", bufs=2, space="PSUM") as scps3:
            emit_scores3 = make_emit_scores(scps3)

            def pop_scores():
                if sq_i[0] < len(squeue):
                    hp, qh, c = squeue[sq_i[0]]
                    sq_i[0] += 1
                    emit_scores3(hp, qh, c)

            with tc.tile_pool(name=f"{R}xps", bufs=3,
                              space="PSUM") as xps, \
                 tc.tile_pool(name=f"{R}pos", bufs=1,
                              space="PSUM") as pos:
                emit_pv_phase = make_pv_phase(xps, make_opart(pos),
                                              pop_scores)
                for hp, qh in ((1, 0), (1, 1), (2, 0), (2, 1)):
                    emit_pv_phase(hp, qh)
            assert sq_i[0] >= len(squeue), "score queue must drain"

        # sub-phases (3,0),(3,1) + opart drain: queue is dry, so the
        # score-pool banks become a deeper opart pipeline
        with tc.tile_pool(name=f"{R}xps2", bufs=4, space="PSUM") as xps2, \
             tc.tile_pool(name=f"{R}pos2", bufs=3, space="PSUM") as pos2:
            opart2 = make_opart(pos2)
            emit_pv_phase2 = make_pv_phase(xps2, opart2, lambda: None)
            for hp, qh in ((3, 0), (3, 1)):
                emit_pv_phase2(hp, qh)
            while backlog:
                ph, pq_ = backlog.pop(0)
                for qt in range(pq_ * 4, pq_ * 4 + 4):
                    for gh in range(2):
                        opart2(ph, qt, gh)
    for cm in reversed(expq):
        cm.__exit__(None, None, None)


def _get_nc(repeat=1):
    if repeat not in _NC:
        _NC[repeat] = _build_nc(repeat)
    return _NC[repeat]


def _mega(x, nblk, bf16):
    """[nblk*128, F] -> [128, nblk*F] with block-major free dim."""
    nb, f = nblk, x.shape[1]
    return np.ascontiguousarray(
        x.reshape(nb, P, f).transpose(1, 0, 2).reshape(P, nb * f)
        .astype(bf16))


def _mega_ft(x, bf16):
    """[NET*128, NFT*128] -> [128, NFT*NET*128], ft-major then j."""
    x4 = x.reshape(NET, P, NFT, P).transpose(1, 2, 0, 3)
    return np.ascontiguousarray(
        x4.reshape(P, NFT * NET * P).astype(bf16))


def make_in_maps(enc, enc_mask, dec, q_w, q_b, k_w, k_b, v_w, v_b, o_w, o_b):
    import ml_dtypes
    bf16 = ml_dtypes.bfloat16
    f32 = np.float32
    ca = np.ascontiguousarray
    in_maps = []
    decT_b, encT_b = [], []
    for b in range(B):
        dT = np.asarray(dec[b], dtype=f32).T          # [1024, 1024]
        decT_b.append(_mega(dT, NET, bf16))
        eT = np.asarray(enc[b], dtype=f32).T          # [1024, 2048]
        # sg-major, then j within: [128, 4*8*512]
        e4 = eT.reshape(NET, P, 4, 512).transpose(1, 2, 0, 3)
        encT_b.append(ca(e4.reshape(P, NET * S_ENC).astype(bf16)))
    for c in range(NCORES):
        b, hg = c // 2, c % 2
        fs = slice(hg * FSH, (hg + 1) * FSH)
        mb = np.where(np.asarray(enc_mask[b, 0, 0]), f32(-1e30), f32(0.0))
        in_maps.append({
            "decT": decT_b[b],
            "encT": encT_b[b],
            "qw": _mega_ft(np.asarray(q_w[:, fs], dtype=f32), bf16),
            "kw": _mega_ft(np.asarray(k_w[:, fs], dtype=f32), bf16),
            "vw": _mega(np.asarray(v_w[:, fs], dtype=f32), NET, bf16),
            "ow": _mega(np.asarray(o_w[fs, :], dtype=f32), NFT, bf16),
            "qb": ca(np.asarray(q_b[fs], dtype=f32).reshape(NFT, P).T),
            "maskb": ca(mb.astype(f32).reshape(NST, P).T),
        })
    return in_maps


def assemble(results, o_b, v_b, o_w):
    ob_eff = (np.asarray(o_b, dtype=np.float64)
              + np.asarray(v_b, dtype=np.float64)
              @ np.asarray(o_w, dtype=np.float64)).astype(np.float32)
    out = np.empty((B, S_DEC, D), dtype=np.float32)
    for b in range(B):
        out[b] = (np.asarray(results[2 * b]["outp"], dtype=np.float32)
                  + np.asarray(results[2 * b + 1]["outp"], dtype=np.float32)
                  + ob_eff)
    return out


def kernel(enc, enc_mask, dec, q_w, q_b, k_w, k_b, v_w, v_b, o_w, o_b):
    from concourse.bass_utils import run_bass_kernel_spmd
    nc = _get_nc()
    in_maps = make_in_maps(enc, enc_mask, dec, q_w, q_b, k_w, k_b,
                           v_w, v_b, o_w, o_b)
    res = run_bass_kernel_spmd(nc, in_maps, list(range(NCORES)))
    return assemble(res.results, o_b, v_b, o_w)
